# revision 1
# baseline (speedup 1.0000x reference)
"""Trainium2 Bass kernel for nn_HANModel (2-layer, 2-relation GAT / HAN).

Strategy (8 NeuronCores, SPMD):
  - Edges partitioned by dst-owner core (50000/8 = 6250 dst nodes per core),
    sorted by dst, bucketed into 128-node blocks, padded to whole 128-edge
    tiles (uniform tile counts across cores so one SPMD program serves all).
  - Per dst-block: one large indirect-DMA gather of bf16 feature rows keyed
    by src, attention scores exp(leaky_relu(el[src]+er[dst])) on ACT, and a
    one-hot matmul (dst-local one-hot built with is_equal against an iota
    row) that segment-sums both the softmax denominator and the
    score-weighted messages into PSUM in a single accumulation group.
    The softmax division is applied once per node, not per edge.
  - Three launches: K0 computes feat1 = x@W1 (+ el/er projections) sharded
    by node; host gathers slices and expands per-edge el/er by index; K1
    does layer-1 edge processing + ELU + feat2 = h1@W2 projections; K2 does
    layer-2 edge processing -> output. Host work between launches is pure
    indexing/concatenation.
"""
import os
import sys
import numpy as np
import ml_dtypes

sys.path.insert(0, '/opt/trn_rl_repo')

from concourse import bass, bacc, mybir
import concourse.tile as tile
from concourse.bass_utils import run_bass_kernel_spmd
from concourse.masks import make_identity

BF16 = ml_dtypes.bfloat16
F32 = np.float32

N = 50000
R = 2
NC = 8
NPC = N // NC            # 6250
NBLK = (NPC + 127) // 128  # 49
NPAD = NBLK * 128        # 6272
P = 128
NEG = 0.2

LAST_HW_NS = None
LAST_HW_PARTS = None
_TRACE = os.environ.get("KERNEL_TRACE", "0") == "1"


# ---------------------------------------------------------------- host prep

def _prep_weights(W, al, ar):
    """W:[Fin,H*D], al/ar:[H,D] -> [Fin, H*D + 2H] fp32 = [feat | wl | wr]."""
    H, D = al.shape
    Wr = W.reshape(W.shape[0], H, D)
    wl = np.einsum('khd,hd->kh', Wr, al)
    wr = np.einsum('khd,hd->kh', Wr, ar)
    return np.ascontiguousarray(
        np.concatenate([W, wl, wr], axis=1).astype(F32))


def _edge_structure(src, dst):
    """Static per-core edge structure (independent of feature values).
    Returns (per_core[c][r] = (e_ids list per block, dloc list per block),
             K[r][j] uniform tile counts)."""
    per_core = [[None] * R for _ in range(NC)]
    for r in range(R):
        owner = dst[r] // NPC
        for c in range(NC):
            sel = np.nonzero(owner == c)[0]
            d = dst[r][sel]
            order = np.argsort(d, kind='stable')
            sel = sel[order]
            dloc = dst[r][sel] - c * NPC
            blk = dloc // 128
            cnts = np.bincount(blk, minlength=NBLK)
            starts = np.concatenate([[0], np.cumsum(cnts)])
            eb, db = [], []
            for j in range(NBLK):
                s, e = starts[j], starts[j + 1]
                eb.append(sel[s:e])
                db.append(dloc[s:e] - j * 128)
            per_core[c][r] = (eb, db)
    K = np.zeros((R, NBLK), dtype=np.int64)
    for r in range(R):
        for c in range(NC):
            cnts = np.array([len(b) for b in per_core[c][r][0]])
            K[r] = np.maximum(K[r], (cnts + 127) // 128)
    K = np.maximum(K, 1)
    return per_core, K


HALF = 32768  # int16 index split point for dma_gather


def _class_split(per_core, src):
    """Uniform (across cores) per-(r,j) tile counts for src<HALF (A) and
    src>=HALF (B) classes."""
    KA = np.zeros((R, NBLK), np.int64)
    KB = np.zeros((R, NBLK), np.int64)
    for r in range(R):
        for c in range(NC):
            eb, _ = per_core[c][r]
            for j in range(NBLK):
                s = src[r][eb[j]]
                nA = int((s < HALF).sum())
                nB = int((s >= HALF).sum())
                KA[r][j] = max(KA[r][j], (nA + 127) // 128)
                KB[r][j] = max(KB[r][j], (nB + 127) // 128)
    KA = np.maximum(KA, 1)
    return KA, KB


def _wrap16(ids):
    """dma_gather index layout: pos i -> [i % 16, i // 16], replicated x8.
    Returns [128, len(ids)//16]."""
    return np.tile(ids.reshape(-1, 16).T, (8, 1)).astype(np.int16)


def _pack_edges(per_core, KA, KB, src, dst, el_full, er_full, H):
    """Per (r, j): edges reordered [A | Apad | B | Bpad]; idx slab = wrapped
    int16 A ids then wrapped (B - HALF) ids; meta slab [128, (1+2H)K] =
    [dstf | el | er] in SBUF layout (edge t*128+p -> col t)."""
    idx_all, meta_all = [], []
    for c in range(NC):
        idx_parts, meta_parts = [], []
        for r in range(R):
            eb, db = per_core[c][r]
            for j in range(NBLK):
                kA, kB = int(KA[r][j]), int(KB[r][j])
                e_ids = eb[j]
                s_all = src[r][e_ids]
                isB = s_all >= HALF
                order = np.argsort(isB, kind='stable')
                e_ids = e_ids[order]
                s_all = s_all[order]
                dl = db[j][order]
                el_e = el_full[r][s_all].astype(F32)
                er_e = er_full[r][dst[r][e_ids]].astype(F32)
                nA = int((~isB).sum())
                nB = len(e_ids) - nA

                def padded(arr, n, k, fillA):
                    pad = k * 128 - n
                    return np.concatenate([arr[:n], np.full(
                        (pad,) + arr.shape[1:], fillA, arr.dtype)])

                sA = padded(s_all[:nA], nA, kA, 0)
                sB = padded(s_all[nA:] - HALF, nB, kB, 0) if kB else \
                    np.zeros(0, s_all.dtype)
                dlp = np.concatenate([padded(dl[:nA], nA, kA, 0),
                                      padded(dl[nA:], nB, kB, 0)])
                elp = np.concatenate(
                    [padded(el_e[:nA], nA, kA, -1e9),
                     padded(el_e[nA:], nB, kB, -1e9)]).astype(F32)
                erp = np.concatenate([padded(er_e[:nA], nA, kA, 0),
                                      padded(er_e[nA:], nB, kB, 0)]).astype(F32)
                k = kA + kB
                s_glob = np.concatenate([sA, sB + HALF]) if kB else sA
                idx_parts.append(
                    s_glob.reshape(k, 128).T.astype(np.int32).ravel())
                dstf = dlp.reshape(k, 128).T.astype(F32)
                elw = elp.reshape(k, 128, H).transpose(1, 0, 2).reshape(128, k * H)
                erw = erp.reshape(k, 128, H).transpose(1, 0, 2).reshape(128, k * H)
                meta_parts.append(
                    np.concatenate([dstf, elw, erw], axis=1).astype(F32).ravel())
        idx_all.append(np.ascontiguousarray(np.concatenate(idx_parts)))
        meta_all.append(np.ascontiguousarray(np.concatenate(meta_parts)))
    return idx_all, meta_all


def _slab_offsets(KA, KB, H):
    """Compile-time offsets into the concatenated idx/meta slabs.
    idx offsets are in int16 elements (A slab then B slab per block)."""
    ioff = np.zeros((R, NBLK), np.int64)
    moff = np.zeros((R, NBLK), np.int64)
    io = mo = 0
    for r in range(R):
        for j in range(NBLK):
            kA, kB = int(KA[r][j]), int(KB[r][j])
            ioff[r][j] = io
            moff[r][j] = mo
            io += 128 * (kA + kB)
            mo += 128 * (1 + 2 * H) * (kA + kB)
    return ioff, moff, io, mo


# ------------------------------------------------------------- bass builders

def _new_nc():
    return bacc.Bacc("TRN2", target_bir_lowering=False, debug=False,
                     num_devices=NC)


def _build_k0():
    """feat1/el1/er1 for this core's node slice.
    in: xT [128, NPAD] f32, wc1 [R, 128, 136] f32
    out: feat1 [R, NPAD, 128] bf16, elr1 [R, NPAD, 8] f32"""
    nc = _new_nc()
    xT = nc.dram_tensor("xT", [P, NPAD], mybir.dt.float32, kind="ExternalInput")
    wc1 = nc.dram_tensor("wc1", [R, P, 136], mybir.dt.float32,
                         kind="ExternalInput")
    feat1 = nc.dram_tensor("feat1", [R, NPAD, 128], mybir.dt.bfloat16,
                           kind="ExternalOutput")
    elr1 = nc.dram_tensor("elr1", [R, NPAD, 8], mybir.dt.float32,
                          kind="ExternalOutput")
    with tile.TileContext(nc) as tc:
        with tc.tile_pool(name="const", bufs=1) as cpool, \
             tc.tile_pool(name="sb", bufs=4) as pool, \
             tc.tile_pool(name="ps", bufs=4, space="PSUM") as psum:
            xT_t = cpool.tile([P, NPAD], mybir.dt.float32)
            nc.sync.dma_start(out=xT_t[:], in_=xT[:])
            wc_t = []
            for r in range(R):
                w = cpool.tile([P, 136], mybir.dt.float32, tag=f"wc{r}")
                nc.sync.dma_start(out=w[:], in_=wc1[r])
                wc_t.append(w)
            for r in range(R):
                for j in range(NBLK):
                    ps = psum.tile([P, 136], mybir.dt.float32)
                    nc.tensor.matmul(ps[:], lhsT=xT_t[:, j * P:(j + 1) * P],
                                     rhs=wc_t[r][:], start=True, stop=True)
                    fb = pool.tile([P, 128], mybir.dt.bfloat16, tag="fb")
                    nc.vector.tensor_copy(out=fb[:], in_=ps[:, 0:128])
                    eb = pool.tile([P, 8], mybir.dt.float32, tag="eb")
                    nc.vector.tensor_copy(out=eb[:], in_=ps[:, 128:136])
                    nc.sync.dma_start(out=feat1[r, j * P:(j + 1) * P, :],
                                      in_=fb[:])
                    nc.sync.dma_start(out=elr1[r, j * P:(j + 1) * P, :],
                                      in_=eb[:])
    nc.compile()
    return nc


def _edge_layer(nc, tc, cpool, pool, psum, feats, idx_d, meta_d,
                KA, KB, ioff, moff, H, D, iota_f, acc_big, gdt):
    """Edge-processing phase shared by K1/K2.
    feats: list of R DRAM handles [N, H*D] of dtype gdt (row = 256B).
    acc_big: [P, NBLK * H*D] f32 tile accumulating sum over relations of
    gat outputs (block j at cols [j*H*D, (j+1)*H*D))."""
    HD = H * D
    MW = H + HD    # matmul rhs width per tile (ex | msg)
    for r in range(R):
        for j in range(NBLK):
            kA, kB = int(KA[r][j]), int(KB[r][j])
            k = kA + kB
            io = int(ioff[r][j])
            idx_t = pool.tile([P, k], mybir.dt.int32, tag="idx")
            nc.sync.dma_start(
                out=idx_t[:],
                in_=idx_d[io:io + P * k].rearrange('(p k) -> p k', p=P))
            mw = (1 + 2 * H) * k
            meta_t = pool.tile([P, mw], mybir.dt.float32, tag="meta")
            nc.sync.dma_start(
                out=meta_t[:],
                in_=meta_d[int(moff[r][j]):int(moff[r][j]) + P * mw]
                .rearrange('(p k) -> p k', p=P))
            G = pool.tile([P, k, HD], gdt, tag="G")
            for t in range(k):
                nc.gpsimd.indirect_dma_start(
                    out=G[:, t, :], out_offset=None, in_=feats[r][:],
                    in_offset=bass.IndirectOffsetOnAxis(
                        ap=idx_t[:, t:t + 1], axis=0))
            # scores: exp(lrelu(el + er))  [P, H*k] f32
            esc = pool.tile([P, H * k], mybir.dt.float32, tag="esc")
            nc.vector.tensor_tensor(
                out=esc[:], in0=meta_t[:, k:k + H * k],
                in1=meta_t[:, k + H * k:k + 2 * H * k],
                op=mybir.AluOpType.add)
            esc2 = pool.tile([P, H * k], mybir.dt.float32, tag="esc2")
            nc.vector.scalar_tensor_tensor(
                out=esc2[:], in0=esc[:], scalar=NEG, in1=esc[:],
                op0=mybir.AluOpType.mult, op1=mybir.AluOpType.max)
            nc.scalar.activation(out=esc2[:], in_=esc2[:],
                                 func=mybir.ActivationFunctionType.Exp)
            # M = [ex | msg] bf16 per tile
            M = pool.tile([P, k * MW], mybir.dt.bfloat16, tag="M")
            M3 = M[:].rearrange('p (k c) -> p k c', c=MW)
            G3 = G[:]
            e3 = esc2[:].rearrange('p (k h) -> p k h', h=H)
            nc.vector.tensor_copy(out=M3[:, :, 0:H], in_=e3[:])
            for h in range(H):
                nc.vector.tensor_tensor(
                    out=M3[:, :, H + h * D:H + (h + 1) * D],
                    in0=G3[:, :, h * D:(h + 1) * D],
                    in1=e3[:, :, h:h + 1].to_broadcast([P, k, D]),
                    op=mybir.AluOpType.mult)
            # one-hot accumulate into PSUM
            accum = psum.tile([P, MW], mybir.dt.float32, tag="accum")
            for t in range(k):
                S = pool.tile([P, P], mybir.dt.bfloat16, tag="S")
                nc.vector.tensor_tensor(
                    out=S[:], in0=meta_t[:, t:t + 1].to_broadcast([P, P]),
                    in1=iota_f[:], op=mybir.AluOpType.is_equal)
                nc.tensor.matmul(accum[:], lhsT=S[:],
                                 rhs=M[:, t * MW:(t + 1) * MW],
                                 start=(t == 0), stop=(t == k - 1))
            # block epilogue: out = msg / max(s, eps), accumulate over r
            sm = pool.tile([P, H], mybir.dt.float32, tag="sm")
            nc.vector.tensor_scalar_max(sm[:], accum[:, 0:H], 1e-30)
            rinv = pool.tile([P, H], mybir.dt.float32, tag="rinv")
            nc.vector.reciprocal(rinv[:], sm[:])
            a3 = accum[:, H:MW].rearrange('p (h d) -> p h d', d=D)
            r3 = rinv[:].rearrange('p h -> p h 1' if False else 'p (h o) -> p h o', o=1)
            dst_sl = acc_big[:, j * HD:(j + 1) * HD] \
                .rearrange('p (h d) -> p h d', d=D)
            if r == 0:
                nc.vector.tensor_tensor(
                    out=dst_sl, in0=a3, in1=r3.to_broadcast([P, H, D]),
                    op=mybir.AluOpType.mult)
            else:
                tmp = pool.tile([P, HD], mybir.dt.float32, tag="tmp")
                t3 = tmp[:].rearrange('p (h d) -> p h d', d=D)
                nc.vector.tensor_tensor(
                    out=t3, in0=a3, in1=r3.to_broadcast([P, H, D]),
                    op=mybir.AluOpType.mult)
                nc.vector.tensor_tensor(
                    out=acc_big[:, j * HD:(j + 1) * HD],
                    in0=acc_big[:, j * HD:(j + 1) * HD], in1=tmp[:],
                    op=mybir.AluOpType.add)


def _build_k1(KA, KB, ioff, moff, itot, mtot):
    """Layer-1 edge processing + ELU + feat2 projections.
    in: feat1_r0/r1 [N,128] bf16; idx1 [itot] i16; meta1 [mtot] f32;
        b1bc [P, NPAD] f32; wc2 [R, 128, 66] f32
    out: feat2 [R, NPAD, 64] f32; elr2 [R, NPAD, 2] f32"""
    nc = _new_nc()
    feats = [nc.dram_tensor(f"feat1_r{r}", [N, 128], mybir.dt.bfloat16,
                            kind="ExternalInput") for r in range(R)]
    idx_d = nc.dram_tensor("idx1", [itot], mybir.dt.int32,
                           kind="ExternalInput")
    meta_d = nc.dram_tensor("meta1", [mtot], mybir.dt.float32,
                            kind="ExternalInput")
    b1bc = nc.dram_tensor("b1bc", [P, NPAD], mybir.dt.float32,
                          kind="ExternalInput")
    wc2 = nc.dram_tensor("wc2", [R, P, 66], mybir.dt.float32,
                         kind="ExternalInput")
    feat2 = nc.dram_tensor("feat2", [R, NPAD, 64], mybir.dt.float32,
                           kind="ExternalOutput")
    elr2 = nc.dram_tensor("elr2", [R, NPAD, 2], mybir.dt.float32,
                          kind="ExternalOutput")
    with tile.TileContext(nc) as tc:
        with tc.tile_pool(name="const", bufs=1) as cpool, \
             tc.tile_pool(name="sb", bufs=3) as pool, \
             tc.tile_pool(name="sS", bufs=6) as spool, \
             tc.tile_pool(name="ps", bufs=2, space="PSUM") as psum:
            iota_i = cpool.tile([P, P], mybir.dt.int32)
            nc.gpsimd.iota(iota_i[:], pattern=[[1, P]], base=0,
                           channel_multiplier=0)
            iota_f = cpool.tile([P, P], mybir.dt.float32)
            nc.vector.tensor_copy(out=iota_f[:], in_=iota_i[:])
            ident = cpool.tile([P, P], mybir.dt.float32)
            make_identity(nc, ident[:])
            b1_t = cpool.tile([P, NPAD], mybir.dt.float32)
            nc.sync.dma_start(out=b1_t[:], in_=b1bc[:])
            wc_t = []
            for r in range(R):
                w = cpool.tile([P, 66], mybir.dt.float32, tag=f"wc{r}")
                nc.sync.dma_start(out=w[:], in_=wc2[r])
                wc_t.append(w)
            h1acc = cpool.tile([P, NPAD], mybir.dt.float32)

            # edge phase writes h1acc (pre-bias gat sum)
            _edge_layer(nc, tc, cpool,
                        _PoolMux(pool, spool), psum, feats, idx_d, meta_d,
                        KA, KB, ioff, moff, 4, 32, iota_f, h1acc,
                        mybir.dt.bfloat16)
            # bias + ELU: h1 = max(g, exp(min(g,0)) - 1)
            nc.vector.tensor_tensor(out=h1acc[:], in0=h1acc[:], in1=b1_t[:],
                                    op=mybir.AluOpType.add)
            t1 = cpool.tile([P, NPAD], mybir.dt.float32)
            nc.vector.tensor_scalar_min(t1[:], h1acc[:], 0.0)
            nc.scalar.activation(out=t1[:], in_=t1[:],
                                 func=mybir.ActivationFunctionType.Exp)
            nc.vector.tensor_scalar_add(t1[:], t1[:], -1.0)
            nc.vector.tensor_tensor(out=h1acc[:], in0=h1acc[:], in1=t1[:],
                                    op=mybir.AluOpType.max)
            # feat2 projections
            for j in range(NBLK):
                psT = psum.tile([P, P], mybir.dt.float32, tag="psT")
                nc.tensor.transpose(out=psT[:],
                                    in_=h1acc[:, j * P:(j + 1) * P],
                                    identity=ident[:])
                h1T = pool.tile([P, P], mybir.dt.float32, tag="h1T")
                nc.vector.tensor_copy(out=h1T[:], in_=psT[:])
                for r in range(R):
                    ps2 = psum.tile([P, 66], mybir.dt.float32, tag="ps2")
                    nc.tensor.matmul(ps2[:], lhsT=h1T[:], rhs=wc_t[r][:],
                                     start=True, stop=True)
                    f2 = pool.tile([P, 64], mybir.dt.float32, tag="f2")
                    nc.vector.tensor_copy(out=f2[:], in_=ps2[:, 0:64])
                    e2 = pool.tile([P, 2], mybir.dt.float32, tag="e2")
                    nc.vector.tensor_copy(out=e2[:], in_=ps2[:, 64:66])
                    nc.sync.dma_start(out=feat2[r, j * P:(j + 1) * P, :],
                                      in_=f2[:])
                    nc.sync.dma_start(out=elr2[r, j * P:(j + 1) * P, :],
                                      in_=e2[:])
    nc.compile()
    return nc


class _PoolMux:
    """Route 'S' tags to a deeper pool, everything else to the main pool."""
    def __init__(self, main, spool):
        self.main = main
        self.spool = spool

    def tile(self, shape, dtype, tag="t"):
        if tag == "S":
            return self.spool.tile(shape, dtype, tag=tag, name=tag)
        return self.main.tile(shape, dtype, tag=tag, name=tag)


def _build_k2(KA, KB, ioff, moff, itot, mtot):
    """Layer-2 edge processing -> y.
    in: feat2_r0/r1 [N,64] f32; idx2 [itot] i16; meta2 [mtot] f32;
        b2bc [P, NBLK*64] f32
    out: y [NPAD, 64] f32"""
    nc = _new_nc()
    feats = [nc.dram_tensor(f"feat2_r{r}", [N, 64], mybir.dt.float32,
                            kind="ExternalInput") for r in range(R)]
    idx_d = nc.dram_tensor("idx2", [itot], mybir.dt.int32,
                           kind="ExternalInput")
    meta_d = nc.dram_tensor("meta2", [mtot], mybir.dt.float32,
                            kind="ExternalInput")
    b2bc = nc.dram_tensor("b2bc", [P, NBLK * 64], mybir.dt.float32,
                          kind="ExternalInput")
    y = nc.dram_tensor("y", [NPAD, 64], mybir.dt.float32,
                       kind="ExternalOutput")
    with tile.TileContext(nc) as tc:
        with tc.tile_pool(name="const", bufs=1) as cpool, \
             tc.tile_pool(name="sb", bufs=3) as pool, \
             tc.tile_pool(name="sS", bufs=6) as spool, \
             tc.tile_pool(name="ps", bufs=2, space="PSUM") as psum:
            iota_i = cpool.tile([P, P], mybir.dt.int32)
            nc.gpsimd.iota(iota_i[:], pattern=[[1, P]], base=0,
                           channel_multiplier=0)
            iota_f = cpool.tile([P, P], mybir.dt.float32)
            nc.vector.tensor_copy(out=iota_f[:], in_=iota_i[:])
            b2_t = cpool.tile([P, NBLK * 64], mybir.dt.float32)
            nc.sync.dma_start(out=b2_t[:], in_=b2bc[:])
            yacc = cpool.tile([P, NBLK * 64], mybir.dt.float32)
            _edge_layer(nc, tc, cpool, _PoolMux(pool, spool), psum,
                        feats, idx_d, meta_d, KA, KB, ioff, moff, 1, 64,
                        iota_f, yacc, mybir.dt.float32)
            nc.vector.tensor_tensor(out=yacc[:], in0=yacc[:], in1=b2_t[:],
                                    op=mybir.AluOpType.add)
            nc.sync.dma_start(
                out=y[:].rearrange('(j p) f -> p j f', p=P),
                in_=yacc[:].rearrange('p (j f) -> p j f', f=64))
    nc.compile()
    return nc


# ------------------------------------------------------------------ runner

def _run(nc, in_maps, label):
    global LAST_HW_PARTS
    if _TRACE:
        try:
            res = run_bass_kernel_spmd(nc, in_maps, core_ids=list(range(NC)),
                                       trace=True)
            if res.exec_time_ns is not None:
                LAST_HW_PARTS[label] = res.exec_time_ns
            return res.results
        except Exception as e:
            print(f"[kernel] trace run failed ({e}); retrying untraced",
                  file=sys.stderr)
    res = run_bass_kernel_spmd(nc, in_maps, core_ids=list(range(NC)))
    return res.results


_PROG_CACHE = {}


def _programs(KA, KB, off1, off2):
    key = (tuple(KA.ravel()), tuple(KB.ravel()))
    if key not in _PROG_CACHE:
        i1, m1, it1, mt1 = off1
        i2, m2, it2, mt2 = off2
        _PROG_CACHE[key] = (
            _build_k0(),
            _build_k1(KA, KB, i1, m1, it1, mt1),
            _build_k2(KA, KB, i2, m2, it2, mt2),
        )
    return _PROG_CACHE[key]


def kernel(x, W1, al1, ar1, b1, W2, al2, ar2, b2, src, dst):
    global LAST_HW_NS, LAST_HW_PARTS
    LAST_HW_PARTS = {}
    x = np.asarray(x, F32)
    src = np.asarray(src, np.int64)
    dst = np.asarray(dst, np.int64)
    W1 = np.asarray(W1, F32); al1 = np.asarray(al1, F32)
    ar1 = np.asarray(ar1, F32); b1 = np.asarray(b1, F32)
    W2 = np.asarray(W2, F32); al2 = np.asarray(al2, F32)
    ar2 = np.asarray(ar2, F32); b2 = np.asarray(b2, F32)

    # static structure
    per_core, _K = _edge_structure(src, dst)
    KA, KB = _class_split(per_core, src)
    off1 = _slab_offsets(KA, KB, 4)
    off2 = _slab_offsets(KA, KB, 1)
    nc0, nc1, nc2 = _programs(KA, KB, off1, off2)

    # ---- K0
    wc1 = np.stack([_prep_weights(W1[r], al1[r], ar1[r]) for r in range(R)])
    xT_slices = []
    for c in range(NC):
        sl = np.zeros((NPAD, 128), F32)
        hi = min(N, c * NPC + NPAD)
        sl[:hi - c * NPC] = x[c * NPC:hi]
        xT_slices.append(np.ascontiguousarray(sl.T))
    in0 = [{"xT": xT_slices[c], "wc1": wc1} for c in range(NC)]
    r0 = _run(nc0, in0, "k0")

    feat1 = np.zeros((R, N, 128), BF16)
    el1 = np.zeros((R, N, 4), F32)
    er1 = np.zeros((R, N, 4), F32)
    for c in range(NC):
        n0, n1 = c * NPC, (c + 1) * NPC
        feat1[:, n0:n1] = r0[c]["feat1"][:, :NPC]
        el1[:, n0:n1] = r0[c]["elr1"][:, :NPC, 0:4]
        er1[:, n0:n1] = r0[c]["elr1"][:, :NPC, 4:8]

    # ---- K1
    idx1, meta1 = _pack_edges(per_core, KA, KB, src, dst, el1, er1, 4)
    b1sum = b1.sum(0).astype(F32)
    b1bc = np.ascontiguousarray(np.tile(b1sum[None, :], (P, NBLK)))
    wc2 = np.stack([_prep_weights(W2[r], al2[r], ar2[r]) for r in range(R)])
    f1c = np.ascontiguousarray(feat1[0]), np.ascontiguousarray(feat1[1])
    in1 = [{"feat1_r0": f1c[0], "feat1_r1": f1c[1], "idx1": idx1[c],
            "meta1": meta1[c], "b1bc": b1bc, "wc2": wc2} for c in range(NC)]
    r1 = _run(nc1, in1, "k1")

    feat2 = np.zeros((R, N, 64), F32)
    el2 = np.zeros((R, N, 1), F32)
    er2 = np.zeros((R, N, 1), F32)
    for c in range(NC):
        n0, n1 = c * NPC, (c + 1) * NPC
        feat2[:, n0:n1] = r1[c]["feat2"][:, :NPC]
        el2[:, n0:n1] = r1[c]["elr2"][:, :NPC, 0:1]
        er2[:, n0:n1] = r1[c]["elr2"][:, :NPC, 1:2]

    # ---- K2
    idx2, meta2 = _pack_edges(per_core, KA, KB, src, dst, el2, er2, 1)
    b2sum = b2.sum(0).astype(F32)
    b2bc = np.ascontiguousarray(np.tile(b2sum[None, :], (P, NBLK)))
    f2c = np.ascontiguousarray(feat2[0]), np.ascontiguousarray(feat2[1])
    in2 = [{"feat2_r0": f2c[0], "feat2_r1": f2c[1], "idx2": idx2[c],
            "meta2": meta2[c], "b2bc": b2bc} for c in range(NC)]
    r2 = _run(nc2, in2, "k2")

    y = np.zeros((N, 64), F32)
    for c in range(NC):
        y[c * NPC:(c + 1) * NPC] = r2[c]["y"][:NPC]
    LAST_HW_NS = (sum(LAST_HW_PARTS.values())
                  if len(LAST_HW_PARTS) == 3 else None)
    return y



# revision 2
# speedup vs baseline: 8.4757x; 8.4757x over previous
"""Trainium2 Bass kernel for nn_HANModel (2-layer, 2-relation GAT / HAN).

Single fused SPMD launch on 8 NeuronCores (the previous 3-launch design
spent ~95% of wall time shipping full feature tables host<->device over
the axon tunnel; this one keeps everything device-resident):

  Phase A: each core projects its 6250-node slice: [feat1|el1|er1] =
           x @ [W1|wl1|wr1] per relation.  feat+el rows -> local DRAM
           table (bf16), er -> local DRAM table (f32).
  CC1:     AllGather the [feat1|el1] tables (core-major concat) so each
           core can gather arbitrary src rows.
  Phase B: edge processing for layer 1.  Edges are partitioned by dst
           owner, sorted by dst, bucketed into 128-node dst blocks with
           uniform (max-over-cores) tile counts so one program serves
           all cores.  Per tile: indirect-DMA gather of [feat|el] rows
           keyed by src, indirect-DMA gather of er rows keyed by local
           dst, scores exp(lrelu(el+er)), and a one-hot matmul that
           segment-sums denominator + weighted messages into PSUM.
  Phase C: bias + ELU on h1, projection to [feat2|el2|er2].
  CC2:     AllGather the [feat2|el2] tables.
  Phase D: layer-2 edge processing -> y (+bias) -> bf16 output.

Host work per call is only slab construction (vectorized, memoized on
the graph bytes) and the single launch.  Wire traffic per call is
~24 MB in / 6.4 MB out vs ~500 MB for the 3-launch design.
"""
import sys
import hashlib
import numpy as np
import ml_dtypes

sys.path.insert(0, '/opt/trn_rl_repo')

from concourse import bass, bacc, mybir
import concourse.tile as tile
from concourse.bass_utils import run_bass_kernel_spmd
from concourse.masks import make_identity

BF16 = ml_dtypes.bfloat16
F32 = np.float32

N = 50000
R = 2
NC = 8
NPC = N // NC            # 6250
NBLK = (NPC + 127) // 128  # 49
NPAD = NBLK * 128        # 6272
P = 128
NEG = 0.2

F1, H1, D1 = 128, 4, 32
F2, H2, D2 = 64, 1, 64
GW1 = F1 + H1            # gathered row width layer 1 (feat|el)
CW1 = F1 + 2 * H1        # projection width layer 1 (feat|el|er)
GW2 = F2 + 2 * H2        # 66: feat|el|er (er unused in gathers, keeps rows 4B-aligned)
MW1 = H1 + F1            # scatter matmul rhs width (ex|msg)
MW2 = H2 + F2

PAD_DLOC = NPC           # local-dst value for padding slots: valid er row,
                         # maps to a nonexistent node in the last block

LAST_HW_NS = None
LAST_HW_PARTS = None


# ---------------------------------------------------------------- host prep

def _prep_weights(W, al, ar):
    """W:[Fin,H*D], al/ar:[H,D] -> [Fin, H*D + 2H] = [feat | wl | wr]."""
    H, D = al.shape
    Wr = W.reshape(W.shape[0], H, D)
    wl = np.einsum('khd,hd->kh', Wr, al)
    wr = np.einsum('khd,hd->kh', Wr, ar)
    return np.ascontiguousarray(
        np.concatenate([W, wl, wr], axis=1).astype(BF16))


def _prep_static(src, dst):
    """Static per-core edge slabs (graph-structure only, feature-free).

    Returns K [R,NBLK] uniform tile counts, off [R,NBLK] slot offsets,
    ITOT total slots, src_sl [NC,ITOT] int32 (src remapped to the
    core-major gathered-table row), dl_sl [NC,ITOT] int16 (r*NPAD +
    local dst; padding slots get r*NPAD+PAD_DLOC).
    Slab layout per (r,j) block: slot (t,p) at off + p*K[r,j] + t.
    """
    E = src.shape[1]
    K = np.zeros((R, NBLK), np.int64)
    keys = []
    for r in range(R):
        owner = dst[r] // NPC
        dl = dst[r] - owner * NPC
        blk = dl >> 7
        key = owner * NBLK + blk
        keys.append((key, owner, dl, blk))
        cnt = np.bincount(key, minlength=NC * NBLK).reshape(NC, NBLK)
        K[r] = np.maximum(-(-cnt.max(axis=0) // 128), 1)
    off = np.zeros((R, NBLK), np.int64)
    o = 0
    for r in range(R):
        for j in range(NBLK):
            off[r, j] = o
            o += 128 * int(K[r, j])
    ITOT = o
    src_sl = np.zeros((NC, ITOT), np.int32)
    dl_pad = np.empty(ITOT, np.int16)
    for r in range(R):
        for j in range(NBLK):
            s = int(off[r, j])
            dl_pad[s:s + 128 * int(K[r, j])] = r * NPAD + PAD_DLOC
    dl_sl = np.tile(dl_pad, (NC, 1))
    for r in range(R):
        key, owner, dl, blk = keys[r]
        order = np.argsort(key, kind='stable')
        ks = key[order]
        starts = np.zeros(NC * NBLK + 1, np.int64)
        np.cumsum(np.bincount(ks, minlength=NC * NBLK), out=starts[1:])
        rank = np.arange(E, dtype=np.int64) - starts[ks]
        t = rank >> 7
        p = rank & 127
        jb = blk[order]
        c = owner[order]
        slot = off[r, jb] + p * K[r, jb] + t
        sg = src[r][order]
        so = sg // NPC
        sremap = so * (R * NPAD) + r * NPAD + (sg - so * NPC)
        flat = c * ITOT + slot
        src_sl.reshape(-1)[flat] = sremap.astype(np.int32)
        dl_sl.reshape(-1)[flat] = (r * NPAD + dl[order]).astype(np.int16)
    return K, off, ITOT, src_sl, dl_sl


# ------------------------------------------------------------- bass builder

def _edge_phase(nc, pool, spool, psum, K, off, r, j, sidx, dl16,
                fglob, erloc, iota_f, GW, FW, H, D, acc_big):
    """One (relation, dst-block) of edge processing; accumulates the
    softmax-normalized gat output into acc_big cols [j*H*D,(j+1)*H*D)."""
    HD = H * D
    MW = H + HD
    k = int(K[r, j])
    io = int(off[r, j])
    idx_t = pool.tile([P, k], mybir.dt.int32, tag="idx", name="idx")
    nc.sync.dma_start(
        out=idx_t[:],
        in_=sidx[io:io + P * k].rearrange('(p k) -> p k', p=P))
    dl_t = pool.tile([P, k], mybir.dt.int16, tag="dl", name="dl")
    nc.sync.dma_start(
        out=dl_t[:],
        in_=dl16[io:io + P * k].rearrange('(p k) -> p k', p=P))
    dl32 = pool.tile([P, k], mybir.dt.int32, tag="dl32", name="dl32")
    nc.vector.tensor_copy(out=dl32[:], in_=dl_t[:])
    dstf = pool.tile([P, k], mybir.dt.float32, tag="dstf", name="dstf")
    nc.vector.tensor_copy(out=dstf[:], in_=dl_t[:])
    nc.vector.tensor_scalar_add(dstf[:], dstf[:], float(-(r * NPAD + j * P)))

    G = pool.tile([P, k, GW], mybir.dt.bfloat16, tag="G", name="G")
    for t in range(k):
        nc.gpsimd.indirect_dma_start(
            out=G[:, t, :], out_offset=None, in_=fglob[:],
            in_offset=bass.IndirectOffsetOnAxis(ap=idx_t[:, t:t + 1], axis=0))
    erE = pool.tile([P, k, 4], mybir.dt.float32, tag="erE", name="erE")
    for t in range(k):
        nc.gpsimd.indirect_dma_start(
            out=erE[:, t, :], out_offset=None, in_=erloc[:],
            in_offset=bass.IndirectOffsetOnAxis(ap=dl32[:, t:t + 1], axis=0))

    esc = pool.tile([P, k * H], mybir.dt.float32, tag="esc", name="esc")
    e3 = esc[:].rearrange('p (k h) -> p k h', h=H)
    nc.vector.tensor_tensor(out=e3, in0=G[:, :, FW:FW + H],
                            in1=erE[:, :, 0:H], op=mybir.AluOpType.add)
    nc.vector.scalar_tensor_tensor(
        out=esc[:], in0=esc[:], scalar=NEG, in1=esc[:],
        op0=mybir.AluOpType.mult, op1=mybir.AluOpType.max)
    nc.scalar.activation(out=esc[:], in_=esc[:],
                         func=mybir.ActivationFunctionType.Exp)

    M = pool.tile([P, k * MW], mybir.dt.bfloat16, tag="M", name="M")
    M3 = M[:].rearrange('p (k c) -> p k c', c=MW)
    nc.vector.tensor_copy(out=M3[:, :, 0:H], in_=e3[:])
    for h in range(H):
        nc.vector.tensor_tensor(
            out=M3[:, :, H + h * D:H + (h + 1) * D],
            in0=G[:, :, h * D:(h + 1) * D],
            in1=e3[:, :, h:h + 1].to_broadcast([P, k, D]),
            op=mybir.AluOpType.mult)

    accum = psum.tile([P, MW], mybir.dt.float32, tag="accum", name="accum")
    for t in range(k):
        S = spool.tile([P, P], mybir.dt.bfloat16, tag="S", name="S")
        nc.vector.tensor_tensor(
            out=S[:], in0=dstf[:, t:t + 1].to_broadcast([P, P]),
            in1=iota_f[:], op=mybir.AluOpType.is_equal)
        nc.tensor.matmul(accum[:], lhsT=S[:], rhs=M[:, t * MW:(t + 1) * MW],
                         start=(t == 0), stop=(t == k - 1))

    sm = pool.tile([P, H], mybir.dt.float32, tag="sm", name="sm")
    nc.vector.tensor_scalar_max(sm[:], accum[:, 0:H], 1e-30)
    rinv = pool.tile([P, H], mybir.dt.float32, tag="rinv", name="rinv")
    nc.vector.reciprocal(rinv[:], sm[:])
    a3 = accum[:, H:MW].rearrange('p (h d) -> p h d', d=D)
    r3 = rinv[:].rearrange('p (h o) -> p h o', o=1)
    dst_sl = acc_big[:, j * HD:(j + 1) * HD].rearrange('p (h d) -> p h d', d=D)
    if r == 0:
        nc.vector.tensor_tensor(out=dst_sl, in0=a3,
                                in1=r3.to_broadcast([P, H, D]),
                                op=mybir.AluOpType.mult)
    else:
        tmp = pool.tile([P, HD], mybir.dt.float32, tag="tmp", name="tmp")
        t3 = tmp[:].rearrange('p (h d) -> p h d', d=D)
        nc.vector.tensor_tensor(out=t3, in0=a3,
                                in1=r3.to_broadcast([P, H, D]),
                                op=mybir.AluOpType.mult)
        nc.vector.tensor_tensor(out=acc_big[:, j * HD:(j + 1) * HD],
                                in0=acc_big[:, j * HD:(j + 1) * HD],
                                in1=tmp[:], op=mybir.AluOpType.add)


def _build_fused(K, off, ITOT):
    nc = bacc.Bacc("TRN2", target_bir_lowering=False, debug=False,
                   num_devices=NC)
    xT = nc.dram_tensor("xT", [P, NPAD], mybir.dt.bfloat16,
                        kind="ExternalInput")
    wc1 = nc.dram_tensor("wc1", [R, P, CW1], mybir.dt.bfloat16,
                         kind="ExternalInput")
    wc2 = nc.dram_tensor("wc2", [R, P, GW2], mybir.dt.bfloat16,
                         kind="ExternalInput")
    b1v = nc.dram_tensor("b1v", [1, F1], mybir.dt.float32,
                         kind="ExternalInput")
    b2v = nc.dram_tensor("b2v", [1, F2], mybir.dt.float32,
                         kind="ExternalInput")
    sidx = nc.dram_tensor("sidx", [ITOT], mybir.dt.int32,
                          kind="ExternalInput")
    dl16 = nc.dram_tensor("dl16", [ITOT], mybir.dt.int16,
                          kind="ExternalInput")
    y = nc.dram_tensor("y", [NPAD, F2], mybir.dt.bfloat16,
                       kind="ExternalOutput")

    f1loc = nc.dram_tensor("f1loc", [R * NPAD, GW1], mybir.dt.bfloat16)
    f1g = nc.dram_tensor("f1g", [NC * R * NPAD, GW1], mybir.dt.bfloat16)
    er1loc = nc.dram_tensor("er1loc", [R * NPAD, 4], mybir.dt.float32)
    f2loc = nc.dram_tensor("f2loc", [R * NPAD, GW2], mybir.dt.bfloat16)
    f2g = nc.dram_tensor("f2g", [NC * R * NPAD, GW2], mybir.dt.bfloat16)
    er2loc = nc.dram_tensor("er2loc", [R * NPAD, 4], mybir.dt.float32)

    with tile.TileContext(nc) as tc:
        with tc.tile_pool(name="const", bufs=1) as cpool:
            iota_i = cpool.tile([P, P], mybir.dt.int32)
            nc.gpsimd.iota(iota_i[:], pattern=[[1, P]], base=0,
                           channel_multiplier=0)
            iota_f = cpool.tile([P, P], mybir.dt.float32)
            nc.vector.tensor_copy(out=iota_f[:], in_=iota_i[:])
            h1acc = cpool.tile([P, NBLK * F1], mybir.dt.float32)
            yacc = cpool.tile([P, NBLK * F2], mybir.dt.float32)

            # ---- Phase A: layer-1 projections of the local node slice
            with tc.tile_pool(name="pa", bufs=1) as apool, \
                 tc.tile_pool(name="pa_w", bufs=4) as wpool, \
                 tc.tile_pool(name="pa_ps", bufs=4, space="PSUM") as apsum:
                xT_t = apool.tile([P, NPAD], mybir.dt.bfloat16)
                nc.sync.dma_start(out=xT_t[:], in_=xT[:])
                wc1_t = []
                for r in range(R):
                    w = apool.tile([P, CW1], mybir.dt.bfloat16,
                                   tag=f"wc1_{r}", name=f"wc1_{r}")
                    nc.sync.dma_start(out=w[:], in_=wc1[r])
                    wc1_t.append(w)
                for j in range(NBLK):
                    for r in range(R):
                        ps = apsum.tile([P, CW1], mybir.dt.float32,
                                        tag="ps", name="ps")
                        nc.tensor.matmul(ps[:],
                                         lhsT=xT_t[:, j * P:(j + 1) * P],
                                         rhs=wc1_t[r][:],
                                         start=True, stop=True)
                        fb = wpool.tile([P, GW1], mybir.dt.bfloat16,
                                        tag="fb", name="fb")
                        nc.vector.tensor_copy(out=fb[:], in_=ps[:, 0:GW1])
                        eb = wpool.tile([P, 4], mybir.dt.float32,
                                        tag="eb", name="eb")
                        nc.vector.tensor_copy(out=eb[:], in_=ps[:, GW1:CW1])
                        row = r * NPAD + j * P
                        nc.sync.dma_start(out=f1loc[row:row + P, :], in_=fb[:])
                        nc.sync.dma_start(out=er1loc[row:row + P, :],
                                          in_=eb[:])

            # ---- CC1: gather all cores' [feat1|el1] tables
            nc.gpsimd.collective_compute(
                "AllGather", mybir.AluOpType.bypass,
                replica_groups=[list(range(NC))],
                ins=[f1loc[:]], outs=[f1g[:]])

            # ---- Phase B: layer-1 edge processing
            with tc.tile_pool(name="pb", bufs=3) as pool, \
                 tc.tile_pool(name="pb_s", bufs=6) as spool, \
                 tc.tile_pool(name="pb_ps", bufs=2, space="PSUM") as psum:
                for r in range(R):
                    for j in range(NBLK):
                        _edge_phase(nc, pool, spool, psum, K, off, r, j,
                                    sidx, dl16, f1g, er1loc, iota_f,
                                    GW1, F1, H1, D1, h1acc)

            # ---- Phase C: bias + ELU, then layer-2 projections
            with tc.tile_pool(name="pc", bufs=1) as cpool2, \
                 tc.tile_pool(name="pc_w", bufs=4) as wpool2, \
                 tc.tile_pool(name="pc_ps", bufs=4, space="PSUM") as psum2:
                b1r = cpool2.tile([1, F1], mybir.dt.float32)
                nc.sync.dma_start(out=b1r[:], in_=b1v[:])
                b1bc = cpool2.tile([P, F1], mybir.dt.float32)
                nc.gpsimd.partition_broadcast(b1bc[:], b1r[:])
                for j in range(NBLK):
                    nc.vector.tensor_tensor(
                        out=h1acc[:, j * F1:(j + 1) * F1],
                        in0=h1acc[:, j * F1:(j + 1) * F1],
                        in1=b1bc[:], op=mybir.AluOpType.add)
                t1 = cpool2.tile([P, NBLK * F1], mybir.dt.float32)
                nc.vector.tensor_scalar_min(t1[:], h1acc[:], 0.0)
                nc.scalar.activation(out=t1[:], in_=t1[:],
                                     func=mybir.ActivationFunctionType.Exp)
                nc.vector.tensor_scalar_add(t1[:], t1[:], -1.0)
                nc.vector.tensor_tensor(out=h1acc[:], in0=h1acc[:],
                                        in1=t1[:], op=mybir.AluOpType.max)
                ident = cpool2.tile([P, P], mybir.dt.float32)
                make_identity(nc, ident[:])
                wc2_t = []
                for r in range(R):
                    w = cpool2.tile([P, GW2], mybir.dt.bfloat16,
                                    tag=f"wc2_{r}", name=f"wc2_{r}")
                    nc.sync.dma_start(out=w[:], in_=wc2[r])
                    wc2_t.append(w)
                for j in range(NBLK):
                    psT = psum2.tile([P, P], mybir.dt.float32,
                                     tag="psT", name="psT")
                    nc.tensor.transpose(out=psT[:],
                                        in_=h1acc[:, j * P:(j + 1) * P],
                                        identity=ident[:])
                    h1T = wpool2.tile([P, P], mybir.dt.bfloat16,
                                      tag="h1T", name="h1T")
                    nc.vector.tensor_copy(out=h1T[:], in_=psT[:])
                    for r in range(R):
                        ps2 = psum2.tile([P, GW2], mybir.dt.float32,
                                         tag="ps2", name="ps2")
                        nc.tensor.matmul(ps2[:], lhsT=h1T[:],
                                         rhs=wc2_t[r][:],
                                         start=True, stop=True)
                        fb2 = wpool2.tile([P, GW2], mybir.dt.bfloat16,
                                          tag="fb2", name="fb2")
                        nc.vector.tensor_copy(out=fb2[:], in_=ps2[:])
                        e1 = wpool2.tile([P, 1], mybir.dt.float32,
                                         tag="e1", name="e1")
                        nc.vector.tensor_copy(out=e1[:],
                                              in_=ps2[:, F2 + 1:F2 + 2])
                        eb2 = wpool2.tile([P, 4], mybir.dt.float32,
                                          tag="eb2", name="eb2")
                        nc.vector.tensor_copy(
                            out=eb2[:], in_=e1[:].to_broadcast([P, 4]))
                        row = r * NPAD + j * P
                        nc.sync.dma_start(out=f2loc[row:row + P, :],
                                          in_=fb2[:])
                        nc.sync.dma_start(out=er2loc[row:row + P, :],
                                          in_=eb2[:])

            # ---- CC2
            nc.gpsimd.collective_compute(
                "AllGather", mybir.AluOpType.bypass,
                replica_groups=[list(range(NC))],
                ins=[f2loc[:]], outs=[f2g[:]])

            # ---- Phase D: layer-2 edge processing
            with tc.tile_pool(name="pd", bufs=3) as pool, \
                 tc.tile_pool(name="pd_s", bufs=6) as spool, \
                 tc.tile_pool(name="pd_ps", bufs=2, space="PSUM") as psum:
                for r in range(R):
                    for j in range(NBLK):
                        _edge_phase(nc, pool, spool, psum, K, off, r, j,
                                    sidx, dl16, f2g, er2loc, iota_f,
                                    GW2, F2, H2, D2, yacc)

            # ---- finalize: + b2, bf16, store
            with tc.tile_pool(name="pf", bufs=1) as fpool:
                b2r = fpool.tile([1, F2], mybir.dt.float32)
                nc.sync.dma_start(out=b2r[:], in_=b2v[:])
                b2bc = fpool.tile([P, F2], mybir.dt.float32)
                nc.gpsimd.partition_broadcast(b2bc[:], b2r[:])
                for j in range(NBLK):
                    nc.vector.tensor_tensor(
                        out=yacc[:, j * F2:(j + 1) * F2],
                        in0=yacc[:, j * F2:(j + 1) * F2],
                        in1=b2bc[:], op=mybir.AluOpType.add)
                yb = fpool.tile([P, NBLK * F2], mybir.dt.bfloat16)
                nc.vector.tensor_copy(out=yb[:], in_=yacc[:])
                nc.sync.dma_start(
                    out=y[:].rearrange('(j p) f -> p j f', p=P),
                    in_=yb[:].rearrange('p (j f) -> p j f', f=F2))
    nc.compile()
    return nc


# ------------------------------------------------------------------ runner

_STATIC_CACHE = {}
_PROG_CACHE = {}


def _static(src, dst):
    h = hashlib.blake2b(src.tobytes(), digest_size=16)
    h.update(dst.tobytes())
    key = h.hexdigest()
    if key not in _STATIC_CACHE:
        _STATIC_CACHE[key] = _prep_static(src, dst)
    return _STATIC_CACHE[key]


def _program(K, off, ITOT):
    key = (tuple(K.ravel()), ITOT)
    if key not in _PROG_CACHE:
        _PROG_CACHE[key] = _build_fused(K, off, ITOT)
    return _PROG_CACHE[key]


def kernel(x, W1, al1, ar1, b1, W2, al2, ar2, b2, src, dst):
    global LAST_HW_NS, LAST_HW_PARTS
    LAST_HW_NS = None
    LAST_HW_PARTS = {}
    x = np.asarray(x, F32)
    src = np.asarray(src, np.int64)
    dst = np.asarray(dst, np.int64)
    W1 = np.asarray(W1, F32); al1 = np.asarray(al1, F32)
    ar1 = np.asarray(ar1, F32); b1 = np.asarray(b1, F32)
    W2 = np.asarray(W2, F32); al2 = np.asarray(al2, F32)
    ar2 = np.asarray(ar2, F32); b2 = np.asarray(b2, F32)

    K, off, ITOT, src_sl, dl_sl = _static(src, dst)
    nc = _program(K, off, ITOT)

    wc1 = np.stack([_prep_weights(W1[r], al1[r], ar1[r]) for r in range(R)])
    wc2 = np.stack([_prep_weights(W2[r], al2[r], ar2[r]) for r in range(R)])
    b1s = np.ascontiguousarray(b1.sum(0)[None, :].astype(F32))
    b2s = np.ascontiguousarray(b2.sum(0)[None, :].astype(F32))
    xb = x.astype(BF16)
    in_maps = []
    for c in range(NC):
        sl = np.zeros((NPAD, P), BF16)
        sl[:NPC] = xb[c * NPC:(c + 1) * NPC]
        in_maps.append({
            "xT": np.ascontiguousarray(sl.T),
            "wc1": wc1, "wc2": wc2, "b1v": b1s, "b2v": b2s,
            "sidx": src_sl[c], "dl16": dl_sl[c],
        })
    res = run_bass_kernel_spmd(nc, in_maps, core_ids=list(range(NC)))
    y = np.zeros((N, F2), F32)
    for c in range(NC):
        y[c * NPC:(c + 1) * NPC] = res.results[c]["y"][:NPC].astype(F32)
    return y


# revision 4
# speedup vs baseline: 41.4620x; 4.8919x over previous
"""Trainium2 Bass kernel for nn_HANModel (2-layer, 2-relation GAT / HAN).

Single fused SPMD launch on 8 NeuronCores, dst-aligned edge layout.

Empirical cost model of this runtime (axon-tunneled PJRT): ~0.1 ms per
engine instruction regardless of size, ~0.8 us per indirect-DMA row
descriptor, ~77 MB/s host->device, ~28 MB/s device->host, ~0.2 s fixed
launch.  The design therefore minimizes instruction and descriptor
counts, wire bytes, and host round-trips:

  - Nodes are RELABELED by total in-degree (descending), striped across
    the 8 cores.  Each core's 6250 nodes form 49 dst blocks of 128;
    partition index = node's slot in its block.
  - Edges are placed dst-ALIGNED: the t-th in-edge of a dst node sits at
    (partition = dst slot, tile = t).  Segment softmax then needs NO
    one-hot matmuls and NO er gather: denominator and message sums are
    plain tensor_reduce over tiles, er is partition-aligned from SBUF.
    Degree sorting makes per-block tile counts track the block's max
    in-degree tightly (~15-25% padding instead of ~80%).
  - Padding slots gather a dedicated PAD ROW of the feature table whose
    el entries are -1e9, so exp(lrelu(el+er)) == 0 masks them with zero
    extra instructions (no bounds_check, which costs ~7 us/descriptor).
  - Phase A projects x -> [feat1|el1] (+er1 kept in SBUF), an AllGather
    shares the tables, layer-1 edge phase, ELU, projection to
    [feat2|el2], second AllGather, layer-2 edge phase, output.
  - The static edge slab (one int32 per slot) is memoized per graph AND
    cached device-resident as a sharded jax.Array, so warm calls ship
    only x + weights (~13 MB) and fetch y (~6.4 MB bf16).
"""
import sys
import hashlib
import numpy as np
import ml_dtypes

sys.path.insert(0, '/opt/trn_rl_repo')

import jax
import jax.numpy as jnp
from jax.sharding import Mesh, PartitionSpec, NamedSharding
from jax.experimental.shard_map import shard_map

from concourse import bass, bacc, mybir
import concourse.tile as tile
from concourse import bass2jax
from concourse.masks import make_identity

BF16 = ml_dtypes.bfloat16
F32 = np.float32

N = 50000
R = 2
NC = 8
NPC = N // NC            # 6250
NBLK = (NPC + 127) // 128  # 49
NPAD = NBLK * 128        # 6272
P = 128
NEG = 0.2

F1, H1, D1 = 128, 4, 32
F2, H2, D2 = 64, 1, 64
GW1 = F1 + H1            # gathered row width layer 1: [feat|el]
CW1 = F1 + 2 * H1        # projection width layer 1: [feat|el|er]
GW2 = F2 + H2            # 65
CW2 = F2 + 2 * H2        # 66
LTAB = R * NPAD + 8      # local table rows (+ pad row at R*NPAD)
PADROW = R * NPAD        # core 0's pad row in the gathered table

LAST_HW_NS = None
LAST_HW_PARTS = None


# ---------------------------------------------------------------- host prep

def _prep_weights(W, al, ar):
    """W:[Fin,H*D], al/ar:[H,D] -> [Fin, H*D + 2H] = [feat | wl | wr]."""
    H, D = al.shape
    Wr = W.reshape(W.shape[0], H, D)
    wl = np.einsum('khd,hd->kh', Wr, al)
    wr = np.einsum('khd,hd->kh', Wr, ar)
    return np.ascontiguousarray(
        np.concatenate([W, wl, wr], axis=1).astype(BF16))


def _prep_static(src, dst):
    """Degree-sorted node relabeling + dst-aligned edge slabs.

    Returns (order, K [R,NBLK], off [R,NBLK], ITOT, slab [NC,ITOT] int32).
    Node at sorted position i lives on core i%NC at slot i//NC.
    Slab layout per (r,j): slot (p,t) at off[r,j] + p*K[r,j] + t, value =
    gathered-table row of the edge's src (or PADROW for padding).
    """
    deg = np.zeros(N, np.int64)
    for r in range(R):
        deg += np.bincount(dst[r], minlength=N)
    order = np.argsort(-deg, kind='stable')
    pc = np.empty(N, np.int64)
    ps = np.empty(N, np.int64)
    ar_ = np.arange(N, dtype=np.int64)
    pc[order] = ar_ % NC
    ps[order] = ar_ // NC

    K = np.zeros((R, NBLK), np.int64)
    indeg = []
    for r in range(R):
        gid = pc[dst[r]] * NPC + ps[dst[r]]
        cnt = np.bincount(gid, minlength=NC * NPC).reshape(NC, NPC)
        cp = np.zeros((NC, NPAD), np.int64)
        cp[:, :NPC] = cnt
        K[r] = np.maximum(cp.reshape(NC, NBLK, 128).max(-1).max(0), 1)
        indeg.append(cnt)
    off = np.zeros((R, NBLK), np.int64)
    o = 0
    for r in range(R):
        for j in range(NBLK):
            off[r, j] = o
            o += 128 * int(K[r, j])
    ITOT = o
    slab = np.full((NC, ITOT), PADROW, np.int32)
    for r in range(R):
        d = dst[r]
        c = pc[d]
        slot = ps[d]
        gid = c * NPC + slot
        o2 = np.argsort(gid, kind='stable')
        gs = gid[o2]
        starts = np.zeros(NC * NPC + 1, np.int64)
        np.cumsum(np.bincount(gs, minlength=NC * NPC), out=starts[1:])
        t = np.arange(len(d), dtype=np.int64) - starts[gs]
        j = (slot[o2]) >> 7
        p = (slot[o2]) & 127
        s = src[r][o2]
        remap = pc[s] * LTAB + r * NPAD + ps[s]
        flat = c[o2] * ITOT + off[r, j] + p * K[r, j] + t
        slab.reshape(-1)[flat] = remap.astype(np.int32)
    return order, K, off, ITOT, slab


# ------------------------------------------------------------- bass builder

def _edge_phase(nc, pool, r, j, k, io, sidx, fglob, er_sb, GW, FW, H, D,
                acc_big):
    """One (relation, dst-block): gather dst-aligned [feat|el] rows,
    scores exp(lrelu(el+er)), reduce denominator+messages over tiles,
    normalize, accumulate into acc_big cols [j*H*D,(j+1)*H*D)."""
    HD = H * D
    idx_t = pool.tile([P, k], mybir.dt.int32, tag="idx", name="idx")
    nc.sync.dma_start(
        out=idx_t[:],
        in_=sidx[io:io + P * k].rearrange('(p k) -> p k', p=P))
    G = pool.tile([P, k, GW], mybir.dt.bfloat16, tag="G", name="G")
    for t in range(k):
        nc.gpsimd.indirect_dma_start(
            out=G[:, t, :], out_offset=None, in_=fglob[:],
            in_offset=bass.IndirectOffsetOnAxis(ap=idx_t[:, t:t + 1], axis=0))
    # scores [P, H, k] (tile axis innermost for reduces)
    esc = pool.tile([P, H, k], mybir.dt.float32, tag="esc", name="esc")
    nc.vector.tensor_tensor(
        out=esc[:], in0=G[:, :, FW:FW + H].rearrange('p k h -> p h k'),
        in1=er_sb.rearrange('p (h o) -> p h o', o=1).to_broadcast([P, H, k]),
        op=mybir.AluOpType.add)
    ef = esc[:].rearrange('p h k -> p (h k)')
    nc.vector.scalar_tensor_tensor(
        out=ef, in0=ef, scalar=NEG, in1=ef,
        op0=mybir.AluOpType.mult, op1=mybir.AluOpType.max)
    nc.scalar.activation(out=ef, in_=ef,
                         func=mybir.ActivationFunctionType.Exp)
    s = pool.tile([P, H], mybir.dt.float32, tag="s", name="s")
    nc.vector.tensor_reduce(out=s[:].rearrange('p (h o) -> p h o', o=1), in_=esc[:],
                            axis=mybir.AxisListType.X, op=mybir.AluOpType.add)
    # messages M [P, H, D, k] = feat * esc
    M = pool.tile([P, HD, k], mybir.dt.bfloat16, tag="M", name="M")
    M4 = M[:].rearrange('p (h d) k -> p h d k', d=D)
    for h in range(H):
        nc.vector.tensor_tensor(
            out=M4[:, h], in0=G[:, :, h * D:(h + 1) * D]
            .rearrange('p k d -> p d k'),
            in1=esc[:, h].rearrange('p (o k) -> p o k', o=1).to_broadcast([P, D, k]),
            op=mybir.AluOpType.mult)
    ms = pool.tile([P, HD], mybir.dt.float32, tag="ms", name="ms")
    nc.vector.tensor_reduce(out=ms[:].rearrange('p (f o) -> p f o', o=1), in_=M[:],
                            axis=mybir.AxisListType.X, op=mybir.AluOpType.add)
    nc.vector.tensor_scalar_max(s[:], s[:], 1e-30)
    rinv = pool.tile([P, H], mybir.dt.float32, tag="rinv", name="rinv")
    nc.vector.reciprocal(rinv[:], s[:])
    m3 = ms[:].rearrange('p (h d) -> p h d', d=D)
    r3 = rinv[:].rearrange('p (h o) -> p h o', o=1)
    dst_sl = acc_big[:, j * HD:(j + 1) * HD].rearrange('p (h d) -> p h d', d=D)
    if r == 0:
        nc.vector.tensor_tensor(out=dst_sl, in0=m3,
                                in1=r3.to_broadcast([P, H, D]),
                                op=mybir.AluOpType.mult)
    else:
        tmp = pool.tile([P, HD], mybir.dt.float32, tag="tmp", name="tmp")
        t3 = tmp[:].rearrange('p (h d) -> p h d', d=D)
        nc.vector.tensor_tensor(out=t3, in0=m3,
                                in1=r3.to_broadcast([P, H, D]),
                                op=mybir.AluOpType.mult)
        nc.vector.tensor_tensor(out=acc_big[:, j * HD:(j + 1) * HD],
                                in0=acc_big[:, j * HD:(j + 1) * HD],
                                in1=tmp[:], op=mybir.AluOpType.add)


def _build_fused(K, off, ITOT):
    nc = bacc.Bacc("TRN2", target_bir_lowering=False, debug=False,
                   num_devices=NC)
    xT = nc.dram_tensor("xT", [P, NPAD], mybir.dt.bfloat16,
                        kind="ExternalInput")
    wc1 = nc.dram_tensor("wc1", [R, P, CW1], mybir.dt.bfloat16,
                         kind="ExternalInput")
    wc2 = nc.dram_tensor("wc2", [R, P, CW2], mybir.dt.bfloat16,
                         kind="ExternalInput")
    b1v = nc.dram_tensor("b1v", [1, F1], mybir.dt.float32,
                         kind="ExternalInput")
    b2v = nc.dram_tensor("b2v", [1, F2], mybir.dt.float32,
                         kind="ExternalInput")
    sidx = nc.dram_tensor("sidx", [ITOT], mybir.dt.int32,
                          kind="ExternalInput")
    y = nc.dram_tensor("y", [NPAD, F2], mybir.dt.bfloat16,
                       kind="ExternalOutput")

    f1loc = nc.dram_tensor("f1loc", [LTAB, GW1], mybir.dt.bfloat16)
    f1g = nc.dram_tensor("f1g", [NC * LTAB, GW1], mybir.dt.bfloat16)
    f2loc = nc.dram_tensor("f2loc", [LTAB, GW2], mybir.dt.bfloat16)
    f2g = nc.dram_tensor("f2g", [NC * LTAB, GW2], mybir.dt.bfloat16)

    with tile.TileContext(nc) as tc:
        with tc.tile_pool(name="const", bufs=1) as cpool:
            h1acc = cpool.tile([P, NBLK * F1], mybir.dt.float32)
            yacc = cpool.tile([P, NBLK * F2], mybir.dt.float32)
            er1_sb = cpool.tile([P, R * NBLK * H1], mybir.dt.float32)
            er2_sb = cpool.tile([P, R * NBLK * H2], mybir.dt.float32)

            # ---- Phase A: layer-1 projections + pad row
            with tc.tile_pool(name="pa", bufs=1) as apool, \
                 tc.tile_pool(name="pa_w", bufs=4) as wpool, \
                 tc.tile_pool(name="pa_ps", bufs=4, space="PSUM") as apsum:
                pad1 = apool.tile([1, GW1], mybir.dt.bfloat16)
                nc.gpsimd.memset(pad1[:], 0.0)
                nc.gpsimd.memset(pad1[:, F1:GW1], -1e9)
                nc.sync.dma_start(out=f1loc[PADROW:PADROW + 1, :],
                                  in_=pad1[:])
                xT_t = apool.tile([P, NPAD], mybir.dt.bfloat16)
                nc.sync.dma_start(out=xT_t[:], in_=xT[:])
                wc1_t = []
                for r in range(R):
                    w = apool.tile([P, CW1], mybir.dt.bfloat16,
                                   tag=f"wc1_{r}", name=f"wc1_{r}")
                    nc.sync.dma_start(out=w[:], in_=wc1[r])
                    wc1_t.append(w)
                for j in range(NBLK):
                    for r in range(R):
                        ps = apsum.tile([P, CW1], mybir.dt.float32,
                                        tag="ps", name="ps")
                        nc.tensor.matmul(ps[:],
                                         lhsT=xT_t[:, j * P:(j + 1) * P],
                                         rhs=wc1_t[r][:],
                                         start=True, stop=True)
                        fb = wpool.tile([P, GW1], mybir.dt.bfloat16,
                                        tag="fb", name="fb")
                        nc.vector.tensor_copy(out=fb[:], in_=ps[:, 0:GW1])
                        nc.scalar.copy(
                            out=er1_sb[:, (r * NBLK + j) * H1:
                                       (r * NBLK + j + 1) * H1],
                            in_=ps[:, GW1:CW1])
                        row = r * NPAD + j * P
                        nc.sync.dma_start(out=f1loc[row:row + P, :], in_=fb[:])

            # ---- CC1
            nc.gpsimd.collective_compute(
                "AllGather", mybir.AluOpType.bypass,
                replica_groups=[list(range(NC))],
                ins=[f1loc[:]], outs=[f1g[:]])

            # ---- Phase B: layer-1 edge processing
            with tc.tile_pool(name="pb", bufs=4) as pool:
                for r in range(R):
                    for j in range(NBLK):
                        _edge_phase(nc, pool, r, j, int(K[r, j]),
                                    int(off[r, j]), sidx, f1g,
                                    er1_sb[:, (r * NBLK + j) * H1:
                                           (r * NBLK + j + 1) * H1],
                                    GW1, F1, H1, D1, h1acc)

            # ---- Phase C: bias + ELU + layer-2 projections + pad row
            with tc.tile_pool(name="pc", bufs=1) as cpool2, \
                 tc.tile_pool(name="pc_w", bufs=4) as wpool2, \
                 tc.tile_pool(name="pc_ps", bufs=4, space="PSUM") as psum2:
                b1r = cpool2.tile([1, F1], mybir.dt.float32)
                nc.sync.dma_start(out=b1r[:], in_=b1v[:])
                b1bc = cpool2.tile([P, F1], mybir.dt.float32)
                nc.gpsimd.partition_broadcast(b1bc[:], b1r[:])
                for j in range(NBLK):
                    nc.vector.tensor_tensor(
                        out=h1acc[:, j * F1:(j + 1) * F1],
                        in0=h1acc[:, j * F1:(j + 1) * F1],
                        in1=b1bc[:], op=mybir.AluOpType.add)
                t1 = cpool2.tile([P, NBLK * F1], mybir.dt.float32)
                nc.vector.tensor_scalar_min(t1[:], h1acc[:], 0.0)
                nc.scalar.activation(out=t1[:], in_=t1[:],
                                     func=mybir.ActivationFunctionType.Exp)
                nc.vector.tensor_scalar_add(t1[:], t1[:], -1.0)
                nc.vector.tensor_tensor(out=h1acc[:], in0=h1acc[:],
                                        in1=t1[:], op=mybir.AluOpType.max)
                pad2 = cpool2.tile([1, GW2], mybir.dt.bfloat16)
                nc.gpsimd.memset(pad2[:], 0.0)
                nc.gpsimd.memset(pad2[:, F2:GW2], -1e9)
                nc.sync.dma_start(out=f2loc[PADROW:PADROW + 1, :],
                                  in_=pad2[:])
                ident = cpool2.tile([P, P], mybir.dt.float32)
                make_identity(nc, ident[:])
                wc2_t = []
                for r in range(R):
                    w = cpool2.tile([P, CW2], mybir.dt.bfloat16,
                                    tag=f"wc2_{r}", name=f"wc2_{r}")
                    nc.sync.dma_start(out=w[:], in_=wc2[r])
                    wc2_t.append(w)
                for j in range(NBLK):
                    psT = psum2.tile([P, P], mybir.dt.float32,
                                     tag="psT", name="psT")
                    nc.tensor.transpose(out=psT[:],
                                        in_=h1acc[:, j * P:(j + 1) * P],
                                        identity=ident[:])
                    h1T = wpool2.tile([P, P], mybir.dt.bfloat16,
                                      tag="h1T", name="h1T")
                    nc.vector.tensor_copy(out=h1T[:], in_=psT[:])
                    for r in range(R):
                        ps2 = psum2.tile([P, CW2], mybir.dt.float32,
                                         tag="ps2", name="ps2")
                        nc.tensor.matmul(ps2[:], lhsT=h1T[:],
                                         rhs=wc2_t[r][:],
                                         start=True, stop=True)
                        fb2 = wpool2.tile([P, GW2], mybir.dt.bfloat16,
                                          tag="fb2", name="fb2")
                        nc.vector.tensor_copy(out=fb2[:], in_=ps2[:, 0:GW2])
                        nc.scalar.copy(
                            out=er2_sb[:, (r * NBLK + j) * H2:
                                       (r * NBLK + j + 1) * H2],
                            in_=ps2[:, GW2:CW2])
                        row = r * NPAD + j * P
                        nc.sync.dma_start(out=f2loc[row:row + P, :],
                                          in_=fb2[:])

            # ---- CC2
            nc.gpsimd.collective_compute(
                "AllGather", mybir.AluOpType.bypass,
                replica_groups=[list(range(NC))],
                ins=[f2loc[:]], outs=[f2g[:]])

            # ---- Phase D: layer-2 edge processing
            with tc.tile_pool(name="pd", bufs=4) as pool:
                for r in range(R):
                    for j in range(NBLK):
                        _edge_phase(nc, pool, r, j, int(K[r, j]),
                                    int(off[r, j]), sidx, f2g,
                                    er2_sb[:, (r * NBLK + j) * H2:
                                           (r * NBLK + j + 1) * H2],
                                    GW2, F2, H2, D2, yacc)

            # ---- finalize
            with tc.tile_pool(name="pf", bufs=1) as fpool:
                b2r = fpool.tile([1, F2], mybir.dt.float32)
                nc.sync.dma_start(out=b2r[:], in_=b2v[:])
                b2bc = fpool.tile([P, F2], mybir.dt.float32)
                nc.gpsimd.partition_broadcast(b2bc[:], b2r[:])
                for j in range(NBLK):
                    nc.vector.tensor_tensor(
                        out=yacc[:, j * F2:(j + 1) * F2],
                        in0=yacc[:, j * F2:(j + 1) * F2],
                        in1=b2bc[:], op=mybir.AluOpType.add)
                yb = fpool.tile([P, NBLK * F2], mybir.dt.bfloat16)
                nc.vector.tensor_copy(out=yb[:], in_=yacc[:])
                nc.sync.dma_start(
                    out=y[:].rearrange('(j p) f -> p j f', p=P),
                    in_=yb[:].rearrange('p (j f) -> p j f', f=F2))
    nc.compile()
    return nc


# ---------------------------------------------- device-cached PJRT runner

class _Runner:
    """Replicates bass2jax.run_bass_via_pjrt's shard_map path but keeps
    designated static inputs device-resident and creates the donated
    zero output buffers on-device."""

    def __init__(self, nc):
        bass2jax.install_neuronx_cc_hook()
        self.nc = nc
        in_names, out_names, out_avals = [], [], []
        pname = nc.partition_id_tensor.name if nc.partition_id_tensor else None
        for alloc in nc.m.functions[0].allocations:
            if not isinstance(alloc, mybir.MemoryLocationSet):
                continue
            name = alloc.memorylocations[0].name
            if alloc.kind == "ExternalInput":
                if name != pname:
                    in_names.append(name)
            elif alloc.kind == "ExternalOutput":
                shape = tuple(alloc.tensor_shape)
                out_names.append(name)
                out_avals.append(
                    jax.core.ShapedArray(shape, mybir.dt.np(alloc.dtype)))
        self.in_names = in_names
        self.out_names = out_names
        self.out_avals = out_avals
        n_params = len(in_names)
        all_in = list(in_names) + list(out_names)
        if pname is not None:
            all_in.append(pname)

        def _body(*args):
            operands = list(args)
            if pname is not None:
                operands.append(bass2jax.partition_id_tensor())
            return tuple(bass2jax._bass_exec_p.bind(
                *operands,
                out_avals=tuple(out_avals),
                in_names=tuple(all_in),
                out_names=tuple(out_names),
                lowering_input_output_aliases=(),
                sim_require_finite=True,
                sim_require_nnan=True,
                nc=nc,
            ))

        devices = jax.devices()[:NC]
        self.mesh = Mesh(np.asarray(devices), ("core",))
        n_outs = len(out_names)
        donate = tuple(range(n_params, n_params + n_outs))
        self.sharded = jax.jit(
            shard_map(_body, mesh=self.mesh,
                      in_specs=(PartitionSpec("core"),) * (n_params + n_outs),
                      out_specs=(PartitionSpec("core"),) * n_outs,
                      check_rep=False),
            donate_argnums=donate, keep_unused=True)
        self.sharding = NamedSharding(self.mesh, PartitionSpec("core"))
        self._zero_fns = [
            jax.jit(lambda a=a: jnp.zeros((NC * a.shape[0], *a.shape[1:]),
                                          a.dtype),
                    out_shardings=self.sharding)
            for a in out_avals]
        self.static = {}     # name -> device-resident concatenated jax.Array

    def put_static(self, name, per_core_arrays):
        self.static[name] = jax.device_put(
            np.concatenate(per_core_arrays, axis=0), self.sharding)

    def run(self, in_maps):
        args = []
        for name in self.in_names:
            if name in self.static:
                args.append(self.static[name])
            else:
                args.append(np.concatenate(
                    [np.asarray(m[name]) for m in in_maps], axis=0))
        zeros = [zf() for zf in self._zero_fns]
        outs = self.sharded(*args, *zeros)
        res = []
        for c in range(NC):
            res.append({
                name: np.asarray(outs[i]).reshape(
                    NC, *self.out_avals[i].shape)[c]
                for i, name in enumerate(self.out_names)})
        return res


# ------------------------------------------------------------------ runner

_STATIC_CACHE = {}
_PROG_CACHE = {}


def _static(src, dst):
    h = hashlib.blake2b(src.tobytes(), digest_size=16)
    h.update(dst.tobytes())
    key = h.hexdigest()
    if key not in _STATIC_CACHE:
        _STATIC_CACHE[key] = _prep_static(src, dst)
    return _STATIC_CACHE[key]


def _program(K, off, ITOT, slab):
    key = (tuple(K.ravel()), ITOT)
    if key not in _PROG_CACHE:
        nc = _build_fused(K, off, ITOT)
        runner = _Runner(nc)
        runner.put_static("sidx", [slab[c] for c in range(NC)])
        _PROG_CACHE[key] = runner
    return _PROG_CACHE[key]


def kernel(x, W1, al1, ar1, b1, W2, al2, ar2, b2, src, dst):
    global LAST_HW_NS, LAST_HW_PARTS
    LAST_HW_NS = None
    LAST_HW_PARTS = {}
    x = np.asarray(x, F32)
    src = np.asarray(src, np.int64)
    dst = np.asarray(dst, np.int64)
    W1 = np.asarray(W1, F32); al1 = np.asarray(al1, F32)
    ar1 = np.asarray(ar1, F32); b1 = np.asarray(b1, F32)
    W2 = np.asarray(W2, F32); al2 = np.asarray(al2, F32)
    ar2 = np.asarray(ar2, F32); b2 = np.asarray(b2, F32)

    order, K, off, ITOT, slab = _static(src, dst)
    runner = _program(K, off, ITOT, slab)

    wc1 = np.stack([_prep_weights(W1[r], al1[r], ar1[r]) for r in range(R)])
    wc2 = np.stack([_prep_weights(W2[r], al2[r], ar2[r]) for r in range(R)])
    b1s = np.ascontiguousarray(b1.sum(0)[None, :].astype(F32))
    b2s = np.ascontiguousarray(b2.sum(0)[None, :].astype(F32))
    xb = x.astype(BF16)
    in_maps = []
    for c in range(NC):
        sl = np.zeros((NPAD, P), BF16)
        sl[:NPC] = xb[order[c::NC]]
        in_maps.append({
            "xT": np.ascontiguousarray(sl.T),
            "wc1": wc1, "wc2": wc2, "b1v": b1s, "b2v": b2s,
        })
    res = runner.run(in_maps)
    y = np.zeros((N, F2), F32)
    for c in range(NC):
        y[order[c::NC]] = res[c]["y"][:NPC].astype(F32)
    return y
